# revision 1
# baseline (speedup 1.0000x reference)
"""Muskingum-Cunge river routing (depth-13 binary tree, N=8191, T=2048) on
8 Trainium2 NeuronCores — parallel-in-time Picard solver.

Idea: per reach, the MC update O_t = C1 I_t + C2 I_{t-1} + C3 O_{t-1} is a
linear recurrence once the (flow-dependent) coefficients are frozen. Solve
each tree level (leaves -> root) for its FULL 2048-step trajectory with the
DVE's native tensor_tensor_scan, then Picard-iterate the coefficients
(K_ITER=3 passes: fixed-point rel err 2.3e-5 vs the sequential reference).
The reference's relu clamp is folded into the fixed point by scanning the
unclamped accumulator U and adding C3*relu(-U_prev) to the scan RHS.

Sharding: core c owns the subtree rooted at the c-th level-3 node (1023
reaches); only the 8 level-3 root trajectories are all-gathered (64 KB),
then every core redundantly solves the tiny 7-node top tree.

Layouts (time along the free dim so the scan works):
  - levels 12..10 (512/256/128 local nodes): node-major [128, T] tiles.
  - levels 9..3 and top 2..0 (<=64 nodes): node-major chunks [128, 2048/k],
    partition = j*k + c; cross-chunk scan carries resolved via a
    PE-transpose to one partition, a masked [1,128] row scan, and a
    transpose back.
Canonical node order ORD (children of canon j at canon j and j+2^l of the
next level) makes every parent-inflow pair-sum a same-partition add: big
levels add two aligned child tiles; small levels store a level's canon
trajectories as [n/2 partitions, 2T] (halves side by side in the free dim)
so the pair-sum is one free-dim-offset tensor_add (the HW requires all
SBUF operands of a compute op to share the same start partition).
"""
import os
import sys

import numpy as np

for _p in ("/opt/trn_rl_repo", "/root/.axon_site/_ro/trn_rl_repo"):
    if os.path.isdir(_p) and _p not in sys.path:
        sys.path.insert(0, _p)

DEPTH = 13
N = 2**DEPTH - 1
T = 2048
NC = 8
K_ITER = 3
F32 = np.float32

CORE_LEVELS = [
    (12, 512, "big"), (11, 256, "big"), (10, 128, "big"),
    (9, 64, "nm"), (8, 32, "nm"), (7, 16, "nm"), (6, 8, "nm"),
    (5, 4, "nm"), (4, 2, "nm"), (3, 1, "nm"),
]
TOP_LEVELS = [(2, 4, "nm"), (1, 2, "nm"), (0, 1, "nm")]
ALL_LEVELS = CORE_LEVELS + TOP_LEVELS
LAT_ROWS = sum(n for _, n, _ in ALL_LEVELS)          # 1030


def _build_ord():
    ORD = [np.array([0], dtype=np.int64)]
    for l in range(DEPTH - 1):
        cur = ORD[l]
        nxt = np.empty(2 * len(cur), dtype=np.int64)
        nxt[: len(cur)] = 2 * cur + 1
        nxt[len(cur):] = 2 * cur + 2
        ORD.append(nxt)
    return ORD


ORD = _build_ord()
NM_LEVELS = [lv for lv, n, kind in ALL_LEVELS if kind == "nm"]


def _level_nodes(core, lv):
    return ORD[lv] if lv < 3 else ORD[lv][core::NC]


def _part_nodes(n, kind):
    return np.arange(128) // (128 // n)


def _host_precompute(inputs):
    lat = np.ascontiguousarray(np.asarray(inputs["lateral_inflows"], F32))
    n_ = np.asarray(inputs["manning_n"], F32).astype(np.float64)
    L = np.asarray(inputs["lengths"], F32).astype(np.float64)
    S = np.asarray(inputs["slopes"], F32).astype(np.float64)
    wc = np.asarray(inputs["width_coefs"], F32).astype(np.float64)
    we = np.asarray(inputs["width_exps"], F32).astype(np.float64)
    dc = np.asarray(inputs["depth_coefs"], F32).astype(np.float64)
    de = np.asarray(inputs["depth_exps"], F32).astype(np.float64)
    c0 = (5.0 / 3.0) * dc ** (2.0 / 3.0) * np.sqrt(S) / n_
    a1n = -(2.0 / 3.0) * de
    a3 = 1.0 - we - (2.0 / 3.0) * de
    ln_half = np.log(0.5)
    P4 = np.log(L / c0) + a1n * ln_half
    P3 = np.log(0.5 / (wc * S * L * c0)) + a3 * ln_half
    consts = np.stack([a1n, a3, P4, P3]).astype(F32)      # [4, N]

    LATs, CONSTs = [], []
    for core in range(NC):
        lat_rows, ccols = [], []
        for lv, n, kind in ALL_LEVELS:
            nodes = _level_nodes(core, lv)
            lat_rows.append(lat[:, nodes].T)
        for lv, n, kind in ALL_LEVELS:
            nodes = _level_nodes(core, lv)
            if kind == "big":
                for ti in range(n // 128):
                    ccols.append(consts[:, nodes[ti * 128:(ti + 1) * 128]])
            else:
                ccols.append(consts[:, nodes[_part_nodes(n, kind)]])
        for lv in NM_LEVELS:
            n = next(n for l, n, k in ALL_LEVELS if l == lv)
            k = 128 // n
            ccols.append((np.arange(128) % k != 0).astype(F32)[None, :])
        dtf = float(inputs["dt"])
        ccols.append(np.full((1, 128), np.log(2.0 * dtf), F32))
        LATs.append(np.ascontiguousarray(np.concatenate(lat_rows, axis=0)))
        CONSTs.append(np.ascontiguousarray(np.concatenate(ccols, axis=0).T.astype(F32)))

    rowmask = []
    for lv in NM_LEVELS:
        n = next(n for l, n, k in ALL_LEVELS if l == lv)
        k = 128 // n
        rowmask.append((np.arange(128) % k != 0).astype(F32))
    ROWMASK = np.ascontiguousarray(np.concatenate(rowmask)[None, :])
    EYE = np.eye(128, dtype=F32)
    return LATs, CONSTs, ROWMASK, EYE


def _const_col(lv, ti=None):
    """Column offset of a (level[, tile])'s 4 constant cols in CONST."""
    off = 0
    for l, n, kind in ALL_LEVELS:
        if kind == "big":
            for t in range(n // 128):
                if l == lv and t == ti:
                    return off
                off += 4
        else:
            if l == lv:
                return off
            off += 4
    raise KeyError((lv, ti))


_NCONST = sum((n // 128) * 4 if k == "big" else 4 for _, n, k in ALL_LEVELS)


def _mask_col(lv):
    return _NCONST + NM_LEVELS.index(lv)


NCOL = _NCONST + len(NM_LEVELS) + 1
_LN2DT_COL = NCOL - 1


def _lat_row(lv):
    off = 0
    for l, n, _ in ALL_LEVELS:
        if l == lv:
            return off
        off += n
    raise KeyError(lv)


def _build_bass(dtf, single=False):
    from contextlib import ExitStack

    import concourse.bass as bass
    import concourse.tile as tile
    from concourse import bacc, mybir

    f32 = mybir.dt.float32
    OP = mybir.AluOpType
    AF = mybir.ActivationFunctionType
    ln2dt = float(np.log(2.0 * dtf))
    inv_dt = 1.0 / dtf

    nc = bacc.Bacc("TRN2", target_bir_lowering=False, debug=False,
                   num_devices=NC)
    lat_d = nc.dram_tensor("lat", [LAT_ROWS, T], f32, kind="ExternalInput").ap()
    const_d = nc.dram_tensor("cst", [128, NCOL], f32, kind="ExternalInput").ap()
    rmask_d = nc.dram_tensor("rmask", [1, len(NM_LEVELS) * 128], f32,
                             kind="ExternalInput").ap()
    eye_d = nc.dram_tensor("eye", [128, 128], f32, kind="ExternalInput").ap()
    out_d = nc.dram_tensor("out", [1, T], f32, kind="ExternalOutput").ap()

    with tile.TileContext(nc) as tc, ExitStack() as ctx:
        cpool = ctx.enter_context(tc.tile_pool(name="const", bufs=1))
        opool = ctx.enter_context(tc.tile_pool(name="lvlO", bufs=6))
        cnpool = ctx.enter_context(tc.tile_pool(name="cn", bufs=2))
        spool = ctx.enter_context(tc.tile_pool(name="scr", bufs=1))
        psum = ctx.enter_context(tc.tile_pool(name="ps", bufs=2, space="PSUM"))
        dram = ctx.enter_context(tc.tile_pool(name="dram", bufs=1, space="DRAM"))

        cst = cpool.tile([128, NCOL], f32)
        nc.sync.dma_start(cst[:], const_d)
        rmask = cpool.tile([1, len(NM_LEVELS) * 128], f32)
        nc.sync.dma_start(rmask[:], rmask_d)
        eye = cpool.tile([128, 128], f32)
        nc.sync.dma_start(eye[:], eye_d)

        def cc(lv, ti=None):
            c0 = _const_col(lv, ti)
            return (cst[:, c0:c0 + 1], cst[:, c0 + 1:c0 + 2],
                    cst[:, c0 + 2:c0 + 3], cst[:, c0 + 3:c0 + 4])

        def emit_solve(Inew, Iold, Lc, kind, consts4, maskcol=None, rmrow=None,
                       Obuf=None):
            """Picard-solve one tile. Inew/Iold: [128, Lc] APs (Iold = I
            shifted by one step, chunk halos included).
            Returns (U, carry) — final O must be relu'd from U by caller."""
            a1c, a3c, P4c, P3c = consts4
            U = spool.tile([128, Lc + 1], f32, tag="U")
            r = spool.tile([128, Lc + 1], f32, tag="r")
            D = spool.tile([128, Lc], f32, tag="D")
            nc.vector.tensor_sub(D[:], Inew, Iold)
            C3b = spool.tile([128, Lc], f32, tag="C3")
            Bb = spool.tile([128, Lc], f32, tag="B")
            carry = None
            if kind != "none":
                carry = spool.tile([128, 1], f32, tag="carry")
            nc.vector.memset(Obuf[:, 0:1], 0.0)
            nc.vector.memset(r[:, 0:1], 0.0)

            for it in range(K_ITER):
                if it == 0:
                    O_old = Iold
                else:
                    nc.scalar.activation(Obuf[:, 1:], U[:, 1:], AF.Relu)
                    nc.gpsimd.tensor_sub(r[:, 1:], Obuf[:, 1:], U[:, 1:])
                    if kind != "none":
                        nc.scalar.activation(Obuf[:, 0:1], carry[:], AF.Relu)
                        nc.gpsimd.tensor_sub(r[:, 0:1], Obuf[:, 0:1], carry[:])
                    O_old = Obuf[:, 0:Lc]
                s1 = spool.tile([128, Lc], f32, tag="s1")
                nc.vector.tensor_add(s1[:], Inew, O_old)
                s2 = spool.tile([128, Lc], f32, tag="s2")
                nc.vector.tensor_scalar(s2[:], s1[:], 2e-3, None, op0=OP.max)
                lq2 = spool.tile([128, Lc], f32, tag="s1")
                nc.scalar.activation(lq2[:], s2[:], AF.Ln)
                K = spool.tile([128, Lc], f32, tag="s3")
                nc.scalar.activation(K[:], lq2[:], AF.Exp, bias=P4c, scale=a1c)
                tt = spool.tile([128, Lc], f32, tag="s2")
                nc.scalar.activation(tt[:], lq2[:], AF.Exp, bias=P3c, scale=a3c)
                w1 = spool.tile([128, Lc], f32, tag="s4")
                nc.gpsimd.tensor_scalar(w1[:], tt[:], 2.0, 1.0, op0=OP.mult,
                                        op1=OP.min)
                v1 = spool.tile([128, Lc], f32, tag="s5")
                nc.vector.tensor_mul(v1[:], K[:], w1[:])
                den = spool.tile([128, Lc], f32, tag="s4")
                nc.vector.scalar_tensor_tensor(den[:], K[:], dtf, v1[:],
                                               OP.add, OP.add)
                lnden = spool.tile([128, Lc], f32, tag="s5")
                nc.scalar.activation(lnden[:], den[:], AF.Ln)
                q2 = spool.tile([128, Lc], f32, tag="s4")
                nc.scalar.activation(q2[:], lnden[:], AF.Exp,
                                     bias=cst[:, _LN2DT_COL:_LN2DT_COL + 1],
                                     scale=-1.0)
                nc.gpsimd.tensor_scalar(C3b[:], q2[:], -1.0, 1.0, op0=OP.mult,
                                        op1=OP.add)
                p2 = spool.tile([128, Lc], f32, tag="s5")
                nc.vector.scalar_tensor_tensor(p2[:], K[:], inv_dt, q2[:],
                                               OP.mult, OP.mult)
                z1 = spool.tile([128, Lc], f32, tag="s3")
                nc.vector.tensor_mul(z1[:], p2[:], D[:])
                m1 = spool.tile([128, Lc], f32, tag="s6")
                nc.vector.tensor_sub(m1[:], D[:], z1[:])
                if it == 0:
                    t2 = spool.tile([128, Lc], f32, tag="s2")
                    nc.gpsimd.tensor_mul(t2[:], q2[:], Iold)
                    nc.vector.tensor_add(Bb[:], m1[:], t2[:])
                else:
                    G = spool.tile([128, Lc], f32, tag="s2")
                    nc.gpsimd.tensor_sub(G[:], r[:, 0:Lc], Iold)
                    z2 = spool.tile([128, Lc], f32, tag="s1")
                    nc.gpsimd.tensor_mul(z2[:], q2[:], G[:])
                    m2 = spool.tile([128, Lc], f32, tag="s3")
                    nc.gpsimd.tensor_add(m2[:], m1[:], r[:, 0:Lc])
                    nc.vector.tensor_sub(Bb[:], m2[:], z2[:])

                if kind == "none":
                    nc.vector.tensor_tensor_scan(U[:, 1:], C3b[:], Bb[:], 0.0,
                                                 OP.mult, OP.add)
                    continue
                Uraw = spool.tile([128, Lc], f32, tag="s6")
                nc.vector.tensor_tensor_scan(Uraw[:], C3b[:], Bb[:], 0.0,
                                             OP.mult, OP.add)
                P = spool.tile([128, Lc], f32, tag="P")
                nc.vector.tensor_tensor_scan(P[:], C3b[:], C3b[:], 1.0,
                                             OP.mult, OP.bypass)
                st = spool.tile([128, 2], f32, tag="st")
                nc.vector.tensor_copy(st[:, 0:1], Uraw[:, Lc - 1:Lc])
                nc.vector.tensor_mul(st[:, 1:2], P[:, Lc - 1:Lc], maskcol)
                psT = psum.tile([1, 256], f32, tag="psT")
                nc.tensor.transpose(psT[0:1, 0:128], st[:, 0:1], eye[:])
                nc.tensor.transpose(psT[0:1, 128:256], st[:, 1:2], eye[:])
                rr = spool.tile([1, 256], f32, tag="rr")
                nc.scalar.copy(rr[:], psT[:])
                crow = spool.tile([1, 128], f32, tag="cr")
                nc.vector.tensor_tensor_scan(crow[:], rr[0:1, 128:256],
                                             rr[0:1, 0:128], 0.0, OP.mult,
                                             OP.add)
                car = spool.tile([1, 128], f32, tag="car")
                nc.vector.memset(car[:, 0:1], 0.0)
                nc.vector.tensor_mul(car[:, 1:128], crow[:, 0:127],
                                     rmrow[:, 1:128])
                psB = psum.tile([128, 1], f32, tag="psB")
                nc.tensor.transpose(psB[:], car[:], eye[0:1, 0:1])
                nc.scalar.copy(carry[:], psB[:])
                nc.vector.scalar_tensor_tensor(U[:, 1:], P[:], carry[:, 0:1],
                                               Uraw[:], OP.mult, OP.add)
            return U, carry

        def emit_chunked_level(lv, n, kind, child_canon, is_top,
                               out_ap=None):
            """Build I, solve, extract. Returns canon tile [n/2, 2T] (node
            halves side by side along the free dim), or writes the flat
            trajectory to out_ap (DRAM [1, T]) when given."""
            k = 128 // n
            Lc = T // k
            r0 = _lat_row(lv)
            slat = spool.tile([128, T], f32, tag="s1")
            nc.sync.dma_start(slat[0:n, :], lat_d[r0:r0 + n, :])
            Icanon = spool.tile([128, T + 1], f32, tag="Icanon")
            nc.vector.memset(Icanon[0:n, 0:1], 0.0)
            sup = spool.tile([128, T], f32, tag="s2")
            nc.vector.tensor_add(sup[0:n, :], child_canon[0:n, 0:T],
                                 child_canon[0:n, T:2 * T])
            nc.vector.tensor_add(Icanon[0:n, 1:], slat[0:n, :], sup[0:n, :])
            # scatter to chunked layout: separate I_new / I_old buffers.
            # All SBUF-side DMA APs stay plain; permutations live on DRAM APs.
            In_t = spool.tile([128, Lc], f32, tag="I")
            Io_t = spool.tile([128, Lc], f32, tag="Io")
            Sg = dram.tile([64, T + 1], f32, tag="Sg")
            nc.sync.dma_start(Sg[0:n, :], Icanon[0:n, :])
            nc.sync.dma_start(
                In_t[:], Sg[0:n, 1:].rearrange("j (c l) -> j c l", c=k))
            nc.sync.dma_start(
                Io_t[:], Sg[0:n, 0:T].rearrange("j (c l) -> j c l", c=k))
            maskcol = cst[:, _mask_col(lv):_mask_col(lv) + 1]
            ri = NM_LEVELS.index(lv) * 128
            rmrow = rmask[:, ri:ri + 128]
            Oc = spool.tile([128, Lc + 1], f32, tag="Oc")
            U, carry = emit_solve(In_t[:], Io_t[:], Lc, kind, cc(lv), maskcol,
                                  rmrow, Obuf=Oc)
            nc.scalar.activation(Oc[:, 1:], U[:, 1:], AF.Relu)
            if out_ap is not None:
                nc.sync.dma_start(
                    out_ap.rearrange("j (c l) -> j c l", c=k), Oc[:, 1:])
                return None
            h = n // 2
            canon = cnpool.tile([64, 2 * T], f32, tag="canon")
            Sx = dram.tile([64, T], f32, tag="Sx")
            nc.sync.dma_start(
                Sx[0:n, :].rearrange("j (c l) -> j c l", c=k), Oc[:, 1:])
            nc.sync.dma_start(canon[0:h, 0:T], Sx[0:h, :])
            nc.sync.dma_start(canon[0:h, T:2 * T], Sx[h:n, :])
            return canon

        # ---- big levels (12, 11, 10) ----
        child_tiles = None
        canon10 = None
        for lv, n, kind in CORE_LEVELS:
            if kind != "big":
                break
            ntile = n // 128
            r0 = _lat_row(lv)
            tiles = []
            for ti in range(ntile):
                Ibig = spool.tile([128, T + 1], f32, tag="I")
                nc.vector.memset(Ibig[:, 0:1], 0.0)
                if lv == 12:
                    nc.sync.dma_start(Ibig[:, 1:],
                                      lat_d[r0 + ti * 128:r0 + (ti + 1) * 128, :])
                else:
                    slat = spool.tile([128, T], f32, tag="s1")
                    nc.sync.dma_start(slat[:],
                                      lat_d[r0 + ti * 128:r0 + (ti + 1) * 128, :])
                    sup = spool.tile([128, T], f32, tag="s2")
                    nc.vector.tensor_add(sup[:], child_tiles[ti][:, 1:],
                                         child_tiles[ti + ntile][:, 1:])
                    nc.vector.tensor_add(Ibig[:, 1:], slat[:], sup[:])
                Obig = opool.tile([128, T + 1], f32, tag="bigO")
                U, _ = emit_solve(Ibig[:, 1:], Ibig[:, 0:T], T, "none", cc(lv, ti), Obuf=Obig)
                nc.scalar.activation(Obig[:, 1:], U[:, 1:], AF.Relu)
                tiles.append(Obig)
            child_tiles = tiles
            if lv == 10:
                canon10 = tiles[0]

        # ---- chunked subtree levels (9..3) ----
        b_in = dram.tile([1, T], f32)
        b_out = dram.tile([NC, T], f32)
        canon10f = cnpool.tile([64, 2 * T], f32, tag="canon")
        nc.sync.dma_start(canon10f[0:64, 0:T], canon10[0:64, 1:])
        nc.sync.dma_start(canon10f[0:64, T:2 * T], canon10[64:128, 1:])
        child_canon = canon10f[:]
        for lv, n, kind in CORE_LEVELS:
            if kind == "big":
                continue
            out_ap = b_in[:] if lv == 3 else None
            canon = emit_chunked_level(lv, n, kind, child_canon, is_top=False,
                                       out_ap=out_ap)
            if canon is not None:
                child_canon = canon[:]
        if single:
            zt = spool.tile([8, T], f32, tag="s1")
            nc.vector.memset(zt[:], 0.0)
            nc.sync.dma_start(b_out[1:8, :], zt[0:7, :])
            nc.sync.dma_start(b_out[0:1, :], b_in[:])
        else:
            nc.gpsimd.collective_compute(
                "AllGather", OP.bypass,
                replica_groups=[list(range(NC))],
                ins=[b_in.opt()], outs=[b_out.opt()])
        roots = cnpool.tile([64, 2 * T], f32, tag="canon")
        nc.sync.dma_start(roots[0:4, 0:T], b_out[0:4, :])
        nc.sync.dma_start(roots[0:4, T:2 * T], b_out[4:8, :])

        # ---- top tree (levels 2..0), replicated on every core ----
        child_canon = roots[:]
        for lv, n, kind in TOP_LEVELS:
            out_ap = out_d if lv == 0 else None
            canon = emit_chunked_level(lv, n, kind, child_canon, is_top=True,
                                       out_ap=out_ap)
            if canon is not None:
                child_canon = canon[:]

    nc.compile()
    return nc


def kernel(**inputs):
    from concourse.bass_utils import run_bass_kernel_spmd

    LATs, CONSTs, ROWMASK, EYE = _host_precompute(inputs)
    dtf = float(inputs["dt"])
    nc = _build_bass(dtf)
    in_maps = [{"lat": LATs[c], "cst": CONSTs[c], "rmask": ROWMASK, "eye": EYE}
               for c in range(NC)]
    res = run_bass_kernel_spmd(nc, in_maps, core_ids=list(range(NC)))
    out = res.results[0]["out"].reshape(-1)
    return out.astype(F32)


if __name__ == "__main__":
    data = np.load("/root/problem/inputs_cache.npz")
    inputs = {k: data[k] for k in data.files}
    out = kernel(**inputs)
    exp = np.load("/root/problem/expected.npy")
    err = np.abs(out - exp) / (np.abs(exp) + 1e-6)
    print("kernel[:4]", out[:4], "expected[:4]", exp[:4])
    print("max rel err", err.max())

